# revision 62
# baseline (speedup 1.0000x reference)
"""LCSA (local convolutional sparse attention) Trainium2 Bass kernel.

Problem: B=2, S=2048, D=1024, H=8 heads, E=128 head width, KW=16 kernel width,
per-head dilations [1,1,2,2,4,4,8,8].

Sharding: pure data-parallel over (batch, sequence): core c handles batch c//4,
sequence chunk (c%4)*512..+512. Each core loads a 640-token haloed slice of x
(64-token halo each side, zero-padded at batch edges; padding reproduces the
reference's "invalid position -> bias" semantics exactly since k(0)=bk, v(0)=bv).

Device algorithm per core (all in fp32):
  - x arrives pre-transposed [D=1024, 640] (host does the transpose for free).
  - qT[h] = (Wq[h].T @ xT) [E,512] and kT[h] [E,640] via PE with W chunks
    stationary; v = xT.T @ Wv_allheads [640, H*E] with xT chunks stationary.
  - Per (query-tile i of 128, head h): logits = qT_tile.T @ kT_window -> [128,256]
    (full 256-key span; additive -30000 mask keeps only the 16 dilated window
    positions), softmax along free dim (DVE max / ACT exp+rowsum / DVE recip),
    score transposed via PE, attnT = v_span.T-chunks @ scoreT, out accumulated
    over heads: out[i] = sum_h attnT[h].T @ Wo[h] (Wo pre-scaled by E**-0.5).
"""

import numpy as np

B, S, D, H, E, KW = 2, 2048, 1024, 8, 128, 16
HALO = 64          # covers max offset d*(KW-1)//2 = 60 for d=8
CHUNK = 512        # query tokens per core
SPAN = CHUNK + 2 * HALO   # 640 = 5*128 kv tokens per core
NST = SPAN // 128  # 5 sequence tiles
NQT = CHUNK // 128 # 4 query tiles
NC_ = 8            # cores
DC = D // 128      # 8 contraction chunks
MASKVAL = -30000.0

_CACHE: dict = {}
_SMBUFS = 3
_SMVBUFS = 6
_PJBUFS = 2
_PSATBUFS = 1
_VFIRST = True
_ATBUFS = 4
_COPY_ENG = "vector"
_P1ENG = "scalar"


def _P1CP(nc, out, in_, bias):
    import concourse.mybir as _mb
    if _P1ENG == "scalar":
        nc.scalar.activation(out, in_, _mb.ActivationFunctionType.Identity,
                             bias=bias, scale=1.0)
    else:
        nc.vector.tensor_scalar_add(out, in_, bias)


def _CP(nc):
    return nc.scalar.copy if _COPY_ENG == "scalar" else nc.vector.tensor_copy


def _build_nc(reps=1, f32r=True):
    from contextlib import ExitStack

    import concourse.bacc as bacc
    import concourse.tile as tile
    from concourse import mybir
    from concourse.masks import make_identity

    F32 = mybir.dt.float32
    # float32r streams the moving matmul operand at 1 cycle/row (vs 4 for
    # plain fp32) once the free dim is >=256; numerically verified against
    # the fp32 reference below.
    FR = mybir.dt.float32r if f32r else F32
    # Scores go through the PE twice more (transpose, then attn matmul with
    # free dim 128 < 256 where fp32r drops to 4 cycles/row). bf16 streams at
    # 1 cycle/row at any free size; post-softmax scores in [0,1] lose ~0.4%.
    BF = mybir.dt.bfloat16
    AX = mybir.AxisListType.X
    AF = mybir.ActivationFunctionType

    nc = bacc.Bacc("TRN2", target_bir_lowering=False, debug=False, num_devices=1)

    xt_d = nc.dram_tensor("xt", [D, SPAN], FR, kind="ExternalInput").ap()
    wq_d = nc.dram_tensor("wq", [H, D, E], FR, kind="ExternalInput").ap()
    wk_d = nc.dram_tensor("wk", [H, D, E], FR, kind="ExternalInput").ap()
    wvr_d = nc.dram_tensor("wvr", [D, H * E], BF, kind="ExternalInput").ap()
    wos_d = nc.dram_tensor("wos", [H, E, D], BF, kind="ExternalInput").ap()
    mk_d = nc.dram_tensor("mk", [H, 128, 256], FR, kind="ExternalInput").ap()
    bqt_d = nc.dram_tensor("bqt", [E, H], F32, kind="ExternalInput").ap()
    bkt_d = nc.dram_tensor("bkt", [E, H], F32, kind="ExternalInput").ap()
    bvr_d = nc.dram_tensor("bvrr", [128, H * E], F32, kind="ExternalInput").ap()
    bor_d = nc.dram_tensor("borr", [128, D], F32, kind="ExternalInput").ap()
    out_d = nc.dram_tensor("out", [CHUNK, D], F32, kind="ExternalOutput").ap()

    with tile.TileContext(nc) as tc, ExitStack() as ctx:
        const_p = ctx.enter_context(tc.tile_pool(name="const", bufs=1))
        big_p = ctx.enter_context(tc.tile_pool(name="big", bufs=1))
        wqk_p = ctx.enter_context(tc.tile_pool(name="wqk", bufs=8))
        sm_p = ctx.enter_context(tc.tile_pool(name="sm", bufs=_SMBUFS))
        smv_p = ctx.enter_context(tc.tile_pool(name="smv", bufs=_SMVBUFS))
        at_p = ctx.enter_context(tc.tile_pool(name="atsb", bufs=_ATBUFS))
        ob_p = ctx.enter_context(tc.tile_pool(name="ob", bufs=2))
        # PSUM banks are 2KB/tile granular (8 total): pj 2 + lg 2 +
        # (st+at merged in one bank) 1 + ou 3 = 8.
        ps_lg = ctx.enter_context(tc.tile_pool(name="ps_lg", bufs=2, space="PSUM"))
        ps_sa = ctx.enter_context(tc.tile_pool(name="ps_sa", bufs=1, space="PSUM"))
        ps_st = ps_sa
        ps_at = ps_sa
        ps_pj = ctx.enter_context(tc.tile_pool(name="ps_pj", bufs=_PJBUFS, space="PSUM"))
        ps_ou = ctx.enter_context(tc.tile_pool(name="ps_ou", bufs=2, space="PSUM"))

        # constants (no DMA: generated on gpsimd)
        ident = const_p.tile([128, 128], BF)
        make_identity(nc, ident)
        ident_f32 = const_p.tile([128, 128], F32)
        make_identity(nc, ident_f32)
        ident_fr = const_p.tile([128, 128], FR)
        nc.vector.tensor_copy(ident_fr, ident_f32)

        for _rep in range(reps):
            # DMA issue order = SP dispatch order: head-0 weights and the
            # per-chunk xt slices first so the PE starts after ~2us, then
            # everything needed later, roughly in first-use order.
            xt_rc = xt_d.rearrange("(c pr) s -> pr c s", pr=128)
            xt_c = []
            # wq0 in halves + lone xt chunk 0: first matmul starts ~4us
            wq0_sb = wqk_p.tile([128, DC, E], FR, tag="wq")
            wq0r = wq_d[0].rearrange("(c p) e -> p c e", p=128)
            nc.sync.dma_start(wq0_sb[:, 0:4, :], wq0r[:, 0:4, :])
            t0 = big_p.tile([128, SPAN], FR, tag="xt0")
            nc.sync.dma_start(t0, xt_rc[:, 0, :])
            xt_c.append(t0)
            nc.sync.dma_start(wq0_sb[:, 4:8, :], wq0r[:, 4:8, :])
            wk0_sb = wqk_p.tile([128, DC, E], FR, tag="wk")
            nc.sync.dma_start(wk0_sb, wk_d[0].rearrange("(c p) e -> p c e", p=128))
            bqt_sb = big_p.tile([128, H], F32, tag="bqt")
            nc.sync.dma_start(bqt_sb, bqt_d)
            bkt_sb = big_p.tile([128, H], F32, tag="bkt")
            nc.sync.dma_start(bkt_sb, bkt_d)
            for cp in range(3):
                t = big_p.tile([128, 2, SPAN], FR, tag=f"xt{cp + 1}")
                nc.sync.dma_start(t, xt_rc[:, 2 * cp + 1:2 * cp + 3, :])
                xt_c.append(t[:, 0, :])
                xt_c.append(t[:, 1, :])
            t7 = big_p.tile([128, SPAN], FR, tag="xt4")
            nc.sync.dma_start(t7, xt_rc[:, 7, :])
            xt_c.append(t7)
            # all remaining head weights queued consecutively on the SP
            # FIFO: every transfer lands just before its head's deadline
            pre_w = []
            for hp in range(1, H):
                wqn = wqk_p.tile([128, DC, E], FR, tag="wq")
                nc.sync.dma_start(wqn, wq_d[hp].rearrange("(c p) e -> p c e", p=128))
                wkn = wqk_p.tile([128, DC, E], FR, tag="wk")
                nc.sync.dma_start(wkn, wk_d[hp].rearrange("(c p) e -> p c e", p=128))
                pre_w.append((wqn, wkn))
            # bulk late-use tensors: allocated here, DMA'd from the (idle)
            # DVE queue inside the head loop so they don't delay the
            # startup-critical SP-queue transfers above.
            wvr_sb = big_p.tile([128, DC, H * E], BF, tag="wvr")
            bvr_sb = big_p.tile([128, H * E], F32, tag="bvr")
            mk_sb = big_p.tile([128, H, 256], FR, tag="mk")
            wos_sb = big_p.tile([128, H, D], BF, tag="wos")
            bor_sb = big_p.tile([128, D], F32, tag="bor")

            def _bulk_dma(h):
                # single emission at h==0: the SP FIFO serializes these
                # after all weight transfers, in deadline order
                if h != 0:
                    return
                for c2 in range(0, DC, 2):
                    nc.sync.dma_start(
                        wvr_sb[:, c2:c2 + 2, :],
                        wvr_d.rearrange("(c p) n -> p c n", p=128)[:, c2:c2 + 2, :])
                nc.sync.dma_start(bvr_sb, bvr_d)
                mk_r = mk_d.rearrange("h p t -> p h t")
                nc.sync.dma_start(mk_sb[:, 0:4, :], mk_r[:, 0:4, :])
                nc.sync.dma_start(mk_sb[:, 4:8, :], mk_r[:, 4:8, :])
                wos_r = wos_d.rearrange("h e d -> e h d")
                for h2 in range(0, H, 2):
                    nc.sync.dma_start(wos_sb[:, h2:h2 + 2, :],
                                      wos_r[:, h2:h2 + 2, :])
                nc.sync.dma_start(bor_sb, bor_d)

            # persistent projection outputs
            qT_sb = big_p.tile([128, H, CHUNK], FR, tag="qT")   # [e, h, s]
            kT_sb = big_p.tile([128, H, SPAN], FR, tag="kT")    # [e, h, s]
            v_sb = []
            for vj in range(NST):
                vt = big_p.tile([128, H * E], BF, tag=f"v{vj}")
                v_sb.append(vt)

            _emit_body(nc, tc, mybir, F32, FR, AX, AF,
                       wq_d, wk_d, out_d, wqk_p, sm_p, smv_p, at_p, ob_p,
                       ps_pj, ps_lg, ps_st, ps_at, ps_ou,
                       ident, ident_fr, xt_c, bqt_sb, bkt_sb, bvr_sb, bor_sb,
                       mk_sb, wvr_sb, wos_sb, qT_sb, kT_sb, v_sb,
                       wq0_sb, wk0_sb, _bulk_dma, pre_w, big_p)

    nc.compile()
    return nc


def _emit_vj(nc, ps_pj, xt_bf, wvr_sb, bvr_sb, v_sb, F32, j):
    # v projection for sequence tile j, all heads (bf16 xT chunks stationary)
    for half in range(2):
        vp = ps_pj.tile([128, 512], F32, tag="pj")
        nsl = slice(512 * half, 512 * (half + 1))
        for c in range(DC):
            nc.tensor.matmul(vp, xt_bf[c][:, 128 * j:128 * (j + 1)],
                             wvr_sb[:, c, nsl], start=(c == 0),
                             stop=(c == DC - 1))
        nc.vector.tensor_add(v_sb[j][:, nsl], vp, bvr_sb[:, nsl])


def _emit_body(nc, tc, mybir, F32, FR, AX, AF,
               wq_d, wk_d, out_d, wqk_p, sm_p, smv_p, at_p, ob_p,
               ps_pj, ps_lg, ps_st, ps_at, ps_ou,
               ident, ident_fr, xt_c, bqt_sb, bkt_sb, bvr_sb, bor_sb,
               mk_sb, wvr_sb, wos_sb, qT_sb, kT_sb, v_sb,
               wq0_sb, wk0_sb, _bulk_dma, pre_w, big_p):
        BF = mybir.dt.bfloat16
        # bf16 copy of xT for the v projection (wvr is bf16; matmul operands
        # must both be 16-bit). DVE is idle during phase 1; the list scheduler
        # runs these as soon as each xt chunk lands.
        xt_bf = []
        for c in range(DC):
            t = big_p.tile([128, SPAN], BF, tag=f"xtb{c}")
            nc.vector.tensor_copy(t, xt_c[c])
            xt_bf.append(t)
        # ---- phase 1a: q/k projections per head (W chunks stationary);
        # weights prefetch 2-3 heads ahead of compute ----
        wq_sb, wk_sb = wq0_sb, wk0_sb
        for h in range(H):
            _bulk_dma(h)

            qp = ps_pj.tile([128, 512], F32, tag="pj")
            for c in range(DC):
                nc.tensor.matmul(qp, wq_sb[:, c, :], xt_c[c][:, HALO:HALO + CHUNK],
                                 start=(c == 0), stop=(c == DC - 1))
            _P1CP(nc, qT_sb[:, h, :], qp, bqt_sb[:, h:h + 1])

            for half in range(2):
                kp = ps_pj.tile([128, 512], F32, tag="pj")
                sl = slice(320 * half, 320 * (half + 1))
                for c in range(DC):
                    nc.tensor.matmul(kp[:, 0:320], wk_sb[:, c, :], xt_c[c][:, sl],
                                     start=(c == 0), stop=(c == DC - 1))
                _P1CP(nc, kT_sb[:, h, sl], kp[:, 0:320], bkt_sb[:, h:h + 1])
            if h + 1 < H:
                wq_sb, wk_sb = pre_w.pop(0)

        # v tiles 0,1 are needed by the first attention steps; the rest
        # interleave into the step loop below, where the list scheduler uses
        # them to fill softmax-chain stalls on the PE.
        _emit_vj(nc, ps_pj, xt_bf, wvr_sb, bvr_sb, v_sb, F32, 0)
        _emit_vj(nc, ps_pj, xt_bf, wvr_sb, bvr_sb, v_sb, F32, 1)

        # ---- phase 2: attention + output projection, software-pipelined
        # two (i,h)-steps ahead so the DVE/ACT softmax chain (~1.6us) hides
        # behind PE work. Mask is added on the PE (identity x mask accumulated
        # into the logits psum); exp reads PSUM directly; no max-subtraction
        # (|logit| <= ~65 so exp stays in fp32 range; masked lanes underflow
        # to exactly 0). ----
        steps = [(i, h) for i in range(NQT) for h in range(H)]
        LOOK = 2    # lg/softmax runs 2 steps ahead of st/at
        LOOKC = 4   # ou consumes ats a full step after it was copied
        sc_t, ou_t, ats_t = {}, {}, {}

        def emit_front(t):
            i, h = steps[t]
            lg = ps_lg.tile([128, 256], F32, tag="lg")
            nc.tensor.matmul(lg, qT_sb[:, h, 128 * i:128 * (i + 1)],
                             kT_sb[:, h, 128 * i:128 * i + 256],
                             start=True, stop=False)
            nc.tensor.matmul(lg, ident_fr, mk_sb[:, h, :], start=False, stop=True)
            ex = sm_p.tile([128, 256], F32, tag="ex")
            se = smv_p.tile([128, 1], F32, tag="se")
            nc.scalar.activation(ex, lg, AF.Exp, bias=0.0, scale=1.0, accum_out=se)
            rc = smv_p.tile([128, 1], F32, tag="rc")
            nc.vector.reciprocal(rc, se)
            sc = sm_p.tile([128, 256], BF, tag="sc")
            nc.vector.tensor_scalar_mul(sc, ex, rc)
            sc_t[t] = sc

        def emit_mid(t):
            i, h = steps[t]
            sc = sc_t.pop(t)
            st = ps_st.tile([128, 256], BF, tag="st")
            nc.tensor.transpose(st[:, 0:128], sc[:, 0:128], ident)
            nc.tensor.transpose(st[:, 128:256], sc[:, 128:256], ident)
            sct = sm_p.tile([128, 256], BF, tag="sct")
            _CP(nc)(sct, st)

            at = ps_at.tile([128, 128], F32, tag="at")
            nc.tensor.matmul(at, v_sb[i][:, E * h:E * (h + 1)], sct[:, 0:128],
                             start=True, stop=False)
            nc.tensor.matmul(at, v_sb[i + 1][:, E * h:E * (h + 1)], sct[:, 128:256],
                             start=False, stop=True)
            ats = at_p.tile([128, 128], BF, tag="ats")
            nc.vector.tensor_copy(ats, at)  # DVE: ACT is busier (exp+accum)
            ats_t[t] = ats

        def emit_back(t):
            i, h = steps[t]
            ats = ats_t.pop(t)
            if h == 0:
                ou0 = ps_ou.tile([128, 512], F32, tag="ou")
                ou1 = ps_ou.tile([128, 512], F32, tag="ou")
                ou_t[i] = (ou0, ou1)
            ou0, ou1 = ou_t[i]
            last = h == H - 1
            nc.tensor.matmul(ou0, ats, wos_sb[:, h, 0:512],
                             start=(h == 0), stop=last)
            nc.tensor.matmul(ou1, ats, wos_sb[:, h, 512:1024],
                             start=(h == 0), stop=last)
            if last:
                # bias folded into the DVE copy-out. Interior tiles: two
                # halves. Final tile: quarters, so adds/transfers/DMA-sems
                # overlap in the kernel tail.
                nq = 2
                w = D // nq
                for qtr in range(nq):
                    cs = slice(w * qtr, w * (qtr + 1))
                    src_ou = (ou0, ou1)[qtr * 2 // nq]
                    off = (w * qtr) % 512
                    obq = ob_p.tile([128, w], F32, tag=f"ob{qtr % 2}")
                    nc.vector.tensor_add(obq, src_ou[:, off:off + w], bor_sb[:, cs])
                    nc.sync.dma_start(out_d[128 * i:128 * (i + 1), cs], obq)
                del ou_t[i]

        for t in range(len(steps) + LOOKC):
            if t < len(steps):
                emit_front(t)
            if t % H == 1 and t // H + 2 < NST:
                _emit_vj(nc, ps_pj, xt_bf, wvr_sb, bvr_sb, v_sb, F32,
                         t // H + 2)
            if LOOK <= t < len(steps) + LOOK:
                emit_mid(t - LOOK)
            if t >= LOOKC:
                emit_back(t - LOOKC)


def _host_prep(x, Wq, bq, Wk, bk, Wv, bv, Wo, bo, dilations):
    import ml_dtypes
    bf = ml_dtypes.bfloat16
    f = np.float32
    x = np.asarray(x, f)
    x_pad = np.zeros((B, S + 2 * HALO, D), f)
    x_pad[:, HALO:HALO + S] = x

    wvr = np.ascontiguousarray(
        np.asarray(Wv, f).transpose(1, 0, 2).reshape(D, H * E)).astype(bf)
    wos = np.ascontiguousarray(
        np.asarray(Wo, f) * np.float32(E) ** f(-0.5)).astype(bf)
    bqt = np.ascontiguousarray(np.asarray(bq, f).T)      # [E, H]
    bkt = np.ascontiguousarray(np.asarray(bk, f).T)
    bvrr = np.ascontiguousarray(
        np.broadcast_to(np.asarray(bv, f).reshape(1, H * E), (128, H * E)))
    borr = np.ascontiguousarray(
        np.broadcast_to(np.asarray(bo, f).reshape(1, D), (128, D)))

    dil = np.asarray(dilations).astype(np.int64)
    masks = np.full((H, 128, 256), MASKVAL, f)
    s_i = np.arange(128)[:, None]
    t_i = np.arange(256)[None, :]
    for h in range(H):
        d = int(dil[h])
        off = (d * (KW - 1)) // 2
        delta = t_i - s_i - HALO + off
        win = (delta >= 0) & (delta <= (KW - 1) * d) & (delta % d == 0)
        masks[h][win] = 0.0

    shared = {
        "wq": np.ascontiguousarray(np.asarray(Wq, f)),
        "wk": np.ascontiguousarray(np.asarray(Wk, f)),
        "wvr": wvr, "wos": wos, "mk": masks,
        "bqt": bqt, "bkt": bkt, "bvrr": bvrr, "borr": borr,
    }
    in_maps = []
    for c in range(NC_):
        b, idx = divmod(c, 4)
        xt = np.ascontiguousarray(x_pad[b, idx * CHUNK: idx * CHUNK + SPAN].T)
        in_maps.append({"xt": xt, **shared})
    return in_maps


def kernel(x, Wq, bq, Wk, bk, Wv, bv, Wo, bo, dilations):
    from concourse.bass_utils import run_bass_kernel_spmd

    if "nc" not in _CACHE:
        _CACHE["nc"] = _build_nc()
    nc = _CACHE["nc"]

    in_maps = _host_prep(x, Wq, bq, Wk, bk, Wv, bv, Wo, bo, dilations)
    res = run_bass_kernel_spmd(nc, in_maps, core_ids=list(range(NC_)))

    out = np.empty((B, S, D), np.float32)
    for c in range(NC_):
        b, idx = divmod(c, 4)
        out[b, idx * CHUNK:(idx + 1) * CHUNK] = res.results[c]["out"]
    return out

